# revision 19
# baseline (speedup 1.0000x reference)
"""Multi-head attention kernel for Trainium2, SPMD over 8 NeuronCores.

Sharding: data-parallel over batch (2 groups of 4 cores) x sequence-parallel
over the key/value length within each group (4 slices of 2048). Each core
computes, for its (batch, k-slice): Q/K/V projections (all heads), masked
softmax numerators/denominators over its k-slice, the attention-weighted
values, and a partial final projection. Denominators are AllReduce'd within
each 4-core group on device (split in two so the first overlaps attention);
the 4 partial projected outputs per batch are summed on the host.

Layout notes: activations/weights are cast to bf16 during the DMA load and
transposed on the TensorE (contraction dims must sit on partitions); scores
are computed transposed ([k, q]) so the exp output is directly consumable as
the stationary operand of the AV matmul; the softmax denominator comes from
a ones-column appended to V; no max-subtraction is needed (scores are O(1)),
and masking is a multiplicative bf16 mask applied after exp (exactly
equivalent to the -1e30 additive mask).
"""

import sys

if "/opt/trn_rl_repo" not in sys.path:
    sys.path.insert(0, "/opt/trn_rl_repo")

from contextlib import ExitStack

import numpy as np

import concourse.bass as bass
import concourse.mybir as mybir
import concourse.tile as tile
from concourse import bacc
from concourse.masks import make_identity

B, QL, KL, D, H = 2, 512, 8192, 1024, 8
HD = D // H  # 128
NCORES = 8
GROUPS = [[0, 1, 2, 3], [4, 5, 6, 7]]
KSH = KL // 4  # 2048 k rows per core
SCALE = 1.0 / float(np.sqrt(HD))

F32 = mybir.dt.float32
BF16 = mybir.dt.bfloat16
U8 = mybir.dt.uint8
P = 128
KC = KSH // P  # 16 k chunks of 128
QB = QL // P  # 4 q blocks


def ensure_ntff_hook():
    """Provide antenv.axon_hooks (missing in this image) so trace=True works.

    Mirrors trn_agent_boot._ntff_profile_via_ctypes against the local
    libaxon_pjrt.so. No-op if the real module exists or the .so is absent.
    """
    try:
        import antenv.axon_hooks  # noqa: F401

        return
    except ImportError:
        pass
    import contextlib
    import ctypes
    import types

    mod = types.ModuleType("antenv.axon_hooks")
    holder = [None]
    mod.set_axon_ntff_profile_hook = lambda h: holder.__setitem__(0, h)
    mod.get_axon_ntff_profile_hook = lambda: holder[0]
    try:
        lib = ctypes.CDLL("/opt/axon/libaxon_pjrt.so")
        if hasattr(lib, "axon_start_nrt_profile"):
            lib.axon_start_nrt_profile.argtypes = [
                ctypes.POINTER(ctypes.c_int64),
                ctypes.c_size_t,
            ]
            lib.axon_start_nrt_profile.restype = ctypes.c_int64
            lib.axon_stop_nrt_profile.argtypes = [ctypes.c_char_p]
            lib.axon_stop_nrt_profile.restype = ctypes.c_int64

            @contextlib.contextmanager
            def _hook(output_dir, device_ids):
                import jax

                jax.devices()
                if device_ids:
                    ids = (ctypes.c_int64 * len(device_ids))(*device_ids)
                    rc = lib.axon_start_nrt_profile(ids, len(device_ids))
                else:
                    rc = lib.axon_start_nrt_profile(None, 0)
                if rc != 0:
                    raise RuntimeError(f"axon_start_nrt_profile rc={rc}")
                try:
                    yield
                finally:
                    n = lib.axon_stop_nrt_profile(str(output_dir).encode())
                    print(f"ntff profile: {n} file(s) -> {output_dir}")

            holder[0] = _hook
    except OSError:
        pass
    sys.modules["antenv.axon_hooks"] = mod
    try:
        import antenv

        antenv.axon_hooks = mod
    except ImportError:
        pass


def build_attention_kernel():
    nc = bacc.Bacc(
        "TRN2", target_bir_lowering=False, debug=False, num_devices=NCORES
    )

    xq = nc.declare_dram_parameter("xq", [QL, D], F32, isOutput=False)
    xk = nc.declare_dram_parameter("xk", [KSH, D], F32, isOutput=False)
    xv = nc.declare_dram_parameter("xv", [KSH, D], F32, isOutput=False)
    msk = nc.declare_dram_parameter("msk", [QL, KSH], U8, isOutput=False)
    wq = nc.declare_dram_parameter("wq", [D, D], F32, isOutput=False)
    wk = nc.declare_dram_parameter("wk", [D, D], F32, isOutput=False)
    wv = nc.declare_dram_parameter("wv", [D, D], F32, isOutput=False)
    wf = nc.declare_dram_parameter("wf", [D, D], F32, isOutput=False)
    out = nc.declare_dram_parameter("out", [QL, D], F32, isOutput=True)

    with tile.TileContext(nc) as tc, ExitStack() as ctx:
        consts = ctx.enter_context(tc.tile_pool(name="consts", bufs=1))
        ident = consts.tile([P, P], BF16)
        make_identity(nc, ident)

        # Persistent operand tiles (single-buffered, live for the kernel).
        persist = ctx.enter_context(tc.tile_pool(name="persist", bufs=1))
        wfT = persist.tile([P, H, D], BF16)  # [din in h-chunk, h, dout]
        kT = persist.tile([P, H, KSH], BF16)  # [hd, head, krow]
        qT = persist.tile([P, H, QL], BF16)  # [hd, head, q]
        v_sb = persist.tile([P, KC, H, HD + 1], BF16)  # [krow, kc, h, hd+1]
        maskT = persist.tile([P, KC, QL], BF16)  # [k, kc, q]
        num_sb = persist.tile([P, H, QB, HD], BF16)  # [q, head, qb, hd]
        den0 = persist.tile([P, 16], F32)  # heads 0-3, [q, (h%4)*4+qb]
        den1 = persist.tile([P, 16], F32)  # heads 4-7
        rden0 = persist.tile([P, 16], F32)
        rden1 = persist.tile([P, 16], F32)
        sumT = persist.tile([P, H, QL], BF16)  # [hd, head, q]

        wts = ctx.enter_context(tc.tile_pool(name="wts", bufs=1))
        loads = ctx.enter_context(tc.tile_pool(name="loads", bufs=3))
        xts = ctx.enter_context(tc.tile_pool(name="xts", bufs=2))
        mn_pool = ctx.enter_context(tc.tile_pool(name="mn_pool", bufs=4))
        probs_pool = ctx.enter_context(tc.tile_pool(name="probs", bufs=3))
        small = ctx.enter_context(tc.tile_pool(name="small", bufs=4))
        outp = ctx.enter_context(tc.tile_pool(name="outp", bufs=2))
        dram = ctx.enter_context(tc.tile_pool(name="dram", bufs=1, space="DRAM"))

        # One PSUM pool, 8 banks: mm 2x2 + av 4x1. Everything except the AV
        # accumulators shares the [128, 2, 512] "mm" slots.
        psum = ctx.enter_context(tc.tile_pool(name="psum", bufs=1, space="PSUM"))

        def mm_tile(name, dtype=F32):
            return psum.tile([P, 2, 512], dtype, tag="mm", bufs=2, name=name)

        def transpose_w(w_dram, dst, wname):
            """dst[p, cc, dout] = w[dout, cc*128+p] (i.e. dst = W^T), bf16."""
            wns = []
            for rg in range(2):
                wn = loads.tile([P, 4, D], BF16, tag="ld", name=f"wn_{wname}{rg}")
                nc.gpsimd.dma_start(
                    out=wn,
                    in_=w_dram[rg * 512 : (rg + 1) * 512, :].rearrange(
                        "(a p) d -> p a d", p=P
                    ),
                )
                wns.append(wn)
            for cc in range(H):
                pst = mm_tile(f"wt_{wname}_{cc}", BF16)
                for rg in range(2):
                    for j in range(4):
                        nc.tensor.transpose(
                            pst[:, rg, j * P : (j + 1) * P],
                            wns[rg][:, j, cc * P : (cc + 1) * P],
                            ident,
                        )
                nc.vector.tensor_copy(
                    dst[:, cc, :], pst[:].rearrange("p a b -> p (a b)")
                )

        # --- Wq, then the Q path ---
        wqT = wts.tile([P, H, D], BF16, tag="wT", name="wqT")
        transpose_w(wq, wqT, "q")

        xqn = loads.tile([P, 4, D], BF16, tag="ld")
        nc.gpsimd.dma_start(out=xqn, in_=xq.rearrange("(a p) d -> p a d", p=P))
        xqT = xts.tile([P, H, QL], BF16, tag="xT")
        for cc2 in range(H // 2):
            pst = mm_tile(f"xqt_{cc2}", BF16)
            for half in range(2):
                cc = cc2 * 2 + half
                for j in range(4):
                    nc.tensor.transpose(
                        pst[:, half, j * P : (j + 1) * P],
                        xqn[:, j, cc * P : (cc + 1) * P],
                        ident,
                    )
            nc.vector.tensor_copy(xqT[:, cc2 * 2 : cc2 * 2 + 2, :], pst[:])
        for m2 in range(H // 2):
            pq = mm_tile(f"pq_{m2}")
            for half in range(2):
                m = m2 * 2 + half
                for cc in range(H):
                    nc.tensor.matmul(
                        pq[:, half, :],
                        wqT[:, cc, m * P : (m + 1) * P],
                        xqT[:, cc, :],
                        start=(cc == 0),
                        stop=(cc == H - 1),
                    )
            nc.any.tensor_copy(out=qT[:, m2 * 2 : m2 * 2 + 2, :], in_=pq[:])

        # --- mask: load+cast per q-block, transpose to [k, q] on PE ---
        mn_tiles = []
        for qb in range(QB):
            mn = mn_pool.tile([P, KSH], BF16, tag="mn", name=f"mn_{qb}")
            nc.gpsimd.dma_start(out=mn, in_=msk[qb * P : (qb + 1) * P, :])
            mn_tiles.append(mn)
        for kc2 in range(KC // 2):
            pst = mm_tile(f"mt_{kc2}", BF16)
            for half in range(2):
                kc = kc2 * 2 + half
                for qb in range(QB):
                    nc.tensor.transpose(
                        pst[:, half, qb * P : (qb + 1) * P],
                        mn_tiles[qb][:, kc * P : (kc + 1) * P],
                        ident,
                    )
            nc.any.tensor_copy(out=maskT[:, kc2 * 2 : kc2 * 2 + 2, :], in_=pst[:])

        # --- Wk, then the K path (stream xk in 512-row chunks) ---
        wkT = wts.tile([P, H, D], BF16, tag="wT", name="wkT")
        transpose_w(wk, wkT, "k")

        for c4 in range(KSH // 512):
            xkn = loads.tile([P, 4, D], BF16, tag="ld", name=f"xkn_{c4}")
            nc.gpsimd.dma_start(
                out=xkn,
                in_=xk[c4 * 512 : (c4 + 1) * 512, :].rearrange("(a p) d -> p a d", p=P),
            )
            xkT = xts.tile([P, H, 512], BF16, tag="xT", name=f"xkT_{c4}")
            for cc2 in range(H // 2):
                pst = mm_tile(f"xkt_{c4}_{cc2}", BF16)
                for half in range(2):
                    cc = cc2 * 2 + half
                    for j in range(4):
                        nc.tensor.transpose(
                            pst[:, half, j * P : (j + 1) * P],
                            xkn[:, j, cc * P : (cc + 1) * P],
                            ident,
                        )
                nc.vector.tensor_copy(xkT[:, cc2 * 2 : cc2 * 2 + 2, :], pst[:])
            for m2 in range(H // 2):
                pk = mm_tile(f"pk_{c4}_{m2}")
                for half in range(2):
                    m = m2 * 2 + half
                    for cc in range(H):
                        nc.tensor.matmul(
                            pk[:, half, :],
                            wkT[:, cc, m * P : (m + 1) * P],
                            xkT[:, cc, :],
                            start=(cc == 0),
                            stop=(cc == H - 1),
                        )
                nc.any.tensor_copy(
                    out=kT[:, m2 * 2 : m2 * 2 + 2, c4 * 512 : (c4 + 1) * 512],
                    in_=pk[:],
                )

        # --- Wv, then the V path ---
        wvT = wts.tile([P, H, D], BF16, tag="wT", name="wvT")
        transpose_w(wv, wvT, "v")

        for c4 in range(KSH // 512):
            xvn = loads.tile([P, 4, D], BF16, tag="ld", name=f"xvn_{c4}")
            nc.gpsimd.dma_start(
                out=xvn,
                in_=xv[c4 * 512 : (c4 + 1) * 512, :].rearrange("(a p) d -> p a d", p=P),
            )
            xvT = xts.tile([P, H, 512], BF16, tag="xT", name=f"xvT_{c4}")
            for cc2 in range(H // 2):
                pst = mm_tile(f"xvt_{c4}_{cc2}", BF16)
                for half in range(2):
                    cc = cc2 * 2 + half
                    for j in range(4):
                        nc.tensor.transpose(
                            pst[:, half, j * P : (j + 1) * P],
                            xvn[:, j, cc * P : (cc + 1) * P],
                            ident,
                        )
                nc.vector.tensor_copy(xvT[:, cc2 * 2 : cc2 * 2 + 2, :], pst[:])
            for mkl in range(4):
                mk = c4 * 4 + mkl
                pv = mm_tile(f"pv_{mk}")
                for n in range(2):
                    for cc in range(H):
                        nc.tensor.matmul(
                            pv[:, n, :],
                            xvT[:, cc, mkl * P : (mkl + 1) * P],
                            wvT[:, cc, n * 512 : (n + 1) * 512],
                            start=(cc == 0),
                            stop=(cc == H - 1),
                        )
                nc.any.tensor_copy(
                    out=v_sb[:, mk, :, 0:HD],
                    in_=pv[:].rearrange("p a (b c) -> p (a b) c", b=4),
                )
        nc.vector.memset(v_sb[:, :, :, HD], 1.0)

        transpose_w(wf, wfT, "f")

        # --- attention per head; exp batched over 2 k-chunks ---
        def attention_head(h, den_tile):
            avs = [
                psum.tile([P, HD + 1], F32, tag="av", bufs=4, name=f"av_{h}_{qb}")
                for qb in range(QB)
            ]
            for kc2 in range(KC // 2):
                ps = mm_tile(f"ps_{h}_{kc2}")
                for half in range(2):
                    kc = kc2 * 2 + half
                    nc.tensor.matmul(
                        ps[:, half, :],
                        kT[:, h, kc * P : (kc + 1) * P],
                        qT[:, h, :],
                        start=True,
                        stop=True,
                    )
                probs = probs_pool.tile(
                    [P, 2, 512], BF16, tag="probs", name=f"pr_{h}_{kc2}"
                )
                nc.scalar.activation(
                    probs[:], ps[:], mybir.ActivationFunctionType.Exp, scale=SCALE
                )
                nc.vector.tensor_mul(
                    probs[:], probs[:], maskT[:, kc2 * 2 : kc2 * 2 + 2, :]
                )
                for half in range(2):
                    kc = kc2 * 2 + half
                    for qb in range(QB):
                        nc.tensor.matmul(
                            avs[qb][:],
                            probs[:, half, qb * P : (qb + 1) * P],
                            v_sb[:, kc, h, :],
                            start=(kc == 0),
                            stop=(kc == KC - 1),
                        )
            for qb in range(QB):
                nc.any.tensor_copy(out=num_sb[:, h, qb, :], in_=avs[qb][:, 0:HD])
                nc.any.tensor_copy(
                    out=den_tile[:, (h % 4) * 4 + qb : (h % 4) * 4 + qb + 1],
                    in_=avs[qb][:, HD : HD + 1],
                )

        def den_allreduce(den_tile, rden_tile, idx):
            den_in = dram.tile([P, 16], F32, name=f"den_in{idx}")
            den_out = dram.tile([P, 16], F32, name=f"den_out{idx}")
            nc.sync.dma_start(out=den_in[:], in_=den_tile[:])
            nc.gpsimd.collective_compute(
                "AllReduce",
                mybir.AluOpType.add,
                replica_groups=GROUPS,
                ins=[den_in.opt()],
                outs=[den_out.opt()],
            )
            nc.sync.dma_start(out=rden_tile[:], in_=den_out[:])
            # guard fully-masked rows (reference wipes them to 0): 0/eps -> 0
            nc.vector.tensor_scalar_max(rden_tile[:], rden_tile[:], 1e-30)
            nc.vector.reciprocal(rden_tile[:], rden_tile[:])

        def norm_head(h, rden_tile):
            snorms = []
            for qb in range(QB):
                snorm = small.tile([P, HD], BF16, tag="snorm", name=f"sn_{h}_{qb}")
                nc.vector.tensor_scalar_mul(
                    snorm[:],
                    num_sb[:, h, qb, :],
                    rden_tile[:, (h % 4) * 4 + qb : (h % 4) * 4 + qb + 1],
                )
                snorms.append(snorm)
            pst = mm_tile(f"st_{h}", BF16)
            for qb in range(QB):
                nc.tensor.transpose(
                    pst[:, 0, qb * P : (qb + 1) * P], snorms[qb][:], ident
                )
            nc.any.tensor_copy(out=sumT[:, h, :], in_=pst[:, 0, :])

        for h in range(4):
            attention_head(h, den0)
        den_allreduce(den0, rden0, 0)
        for h in range(4, H):
            attention_head(h, den1)
        for h in range(4):
            norm_head(h, rden0)
        den_allreduce(den1, rden1, 1)
        for h in range(4, H):
            norm_head(h, rden1)

        for qb2 in range(QB // 2):
            for n in range(2):
                po = mm_tile(f"po_{qb2}_{n}")
                for half in range(2):
                    qb = qb2 * 2 + half
                    for h in range(H):
                        nc.tensor.matmul(
                            po[:, half, :],
                            sumT[:, h, qb * P : (qb + 1) * P],
                            wfT[:, h, n * 512 : (n + 1) * 512],
                            start=(h == 0),
                            stop=(h == H - 1),
                        )
                ot = outp.tile([P, 2, 512], F32, tag="out", name=f"ot_{qb2}_{n}")
                nc.any.tensor_copy(out=ot[:], in_=po[:])
                for half in range(2):
                    qb = qb2 * 2 + half
                    nc.sync.dma_start(
                        out=out[qb * P : (qb + 1) * P, n * 512 : (n + 1) * 512],
                        in_=ot[:, half, :],
                    )

    nc.compile()
    return nc


_NC_CACHE = None


def _get_nc():
    global _NC_CACHE
    if _NC_CACHE is None:
        _NC_CACHE = build_attention_kernel()
    return _NC_CACHE


def make_in_maps(inputs):
    inputs = {k: np.asarray(v) for k, v in inputs.items()}
    in_maps = []
    for c in range(NCORES):
        b, s = c // 4, c % 4
        in_maps.append(
            {
                "xq": np.ascontiguousarray(inputs["inputs_q"][b]),
                "xk": np.ascontiguousarray(
                    inputs["inputs_k"][b, s * KSH : (s + 1) * KSH]
                ),
                "xv": np.ascontiguousarray(
                    inputs["inputs_v"][b, s * KSH : (s + 1) * KSH]
                ),
                "msk": np.ascontiguousarray(
                    inputs["attention_mask"][b, :, s * KSH : (s + 1) * KSH]
                ).view(np.uint8),
                "wq": np.ascontiguousarray(inputs["Wq"]),
                "wk": np.ascontiguousarray(inputs["Wk"]),
                "wv": np.ascontiguousarray(inputs["Wv"]),
                "wf": np.ascontiguousarray(inputs["Wf"]),
            }
        )
    return in_maps


def gather_out(results):
    out = np.zeros((B, QL, D), np.float32)
    for c in range(NCORES):
        out[c // 4] += results[c]["out"]
    return out


def kernel(**inputs) -> np.ndarray:
    ensure_ntff_hook()  # defensive: BASS_TRACE=1 in env would need the shim
    from concourse.bass_utils import run_bass_kernel_spmd

    nc = _get_nc()
    in_maps = make_in_maps(inputs)
    res = run_bass_kernel_spmd(nc, in_maps, list(range(NCORES)))
    return gather_out(res.results)
